# revision 14
# baseline (speedup 1.0000x reference)
"""Trainium2 Bass kernel for nn_MultiHeadAttention_32220844654809.

Mathematical simplification of the reference (faithful to its buggy einsum):

  q/k/v are projected then reshaped to [N,H,S,D].  The einsum
  'nqhd,nkhd->nqhk' contracts only d, so energy[n,hq,s,hk] is a 16x16
  head-head Gram matrix per (n,s); softmax is over hk.  The output einsum
  'nqhk,nvhd->nqhd' has BOTH k and v appearing in only one operand each,
  so it factorizes into (sum_k attention) * (sum_v v) = 1 * vsum, because
  softmax rows sum to one.  Q, K, Wq, Wk drop out entirely.

  vsum[n,s,d] = sum_h v[n,h,s,d] = x[n,s,:] @ Wvsum.T  with
  Wvsum[d,e] = sum_h Wv[h*128+d, e]  (head-summed V projection, [128,2048]).

  out.reshape(N,S,E) is a RAW reshape of the [N,H,S,D] tensor, and since the
  tensor is independent of the leading H axis, the [N,4096,2048] output is 16
  identical copies (along S) of block[n] = vsum[n].reshape(256,2048) @ Wo.T + bo.

Device work per core (data-parallel over batch N=8):
  stage 1: vsumT[d,s] = Wvsum @ x[n].T        (K=E=2048, M=128, N=4096)
  stage 2: block[r,o] = u @ Wo.T              (u = vsum.reshape(256,2048))
where u.T j-tiles are stride-16 views of vsumT in SBUF (no data movement).

Precision: x is shipped as float8_e3m4 (4 mantissa bits) and fed directly to
the PE as the moving operand against an fp16 stationary Wvsum (mixed-dtype
matmul upcasts both to ~fp22 internally).  This halves the dominant DMA
stream (x: 8.4MB instead of 16.8MB/core) so the kernel is PE-bound, while
keeping max|err|/absmax(ref) ~1.3e-2 < 2e-2 (weights/vsum/output stay fp16).
"""

import os
import sys
import types

import numpy as np

import concourse.mybir as mybir
import concourse.tile as tile
from concourse import bacc
from concourse.bass_utils import run_bass_kernel_spmd


def _ensure_ntff_hook():
    """If the image's antenv lacks axon_hooks, synthesize it so
    run_bass_kernel_spmd(trace=True) (e.g. via BASS_TRACE) degrades
    gracefully instead of raising ModuleNotFoundError."""
    try:
        import antenv.axon_hooks  # noqa: F401
        return
    except ImportError:
        pass
    try:
        import antenv
    except ImportError:
        return
    mod = types.ModuleType("antenv.axon_hooks")
    mod._hook = None
    mod.set_axon_ntff_profile_hook = lambda h: setattr(mod, "_hook", h)
    mod.get_axon_ntff_profile_hook = lambda: mod._hook
    sys.modules["antenv.axon_hooks"] = mod
    antenv.axon_hooks = mod
    try:
        from trn_agent_boot.trn_boot import _ntff_profile_via_ctypes
        if os.path.exists("/opt/axon/libaxon_pjrt.so"):
            mod._hook = _ntff_profile_via_ctypes("/opt/axon/libaxon_pjrt.so")
    except Exception:
        mod._hook = None

N_CORES = 8
N, S, E = 8, 4096, 2048
H, D = 16, 128
R = S // H  # 256 distinct output rows per batch; output = this block tiled 16x

F8 = mybir.dt.float8e3
F16 = mybir.dt.float16
F32 = mybir.dt.float32

_prog_cache = {}
last_results = None  # BassKernelResults of the most recent run (for test.py)


def _build_program(s=S):
    """One NeuronCore's program; run SPMD on 8 cores (core n <- batch n)."""
    et_n = E // 128          # 16 contraction tiles (e)
    sc_n = s // 512          # s-chunks of 512
    r = s // H               # block rows
    rt_n = r // 128          # output row tiles (2 at full size)
    oc_n = E // 512          # output col chunks (4)

    nc = bacc.Bacc("TRN2", target_bir_lowering=False, debug=False,
                   num_devices=N_CORES)
    # All DRAM tensors are packed partition-major on host (dim0 = SBUF
    # partition), flat in dim1, so every DMA moves multi-KB contiguous runs
    # per partition and pairs of e-tiles ship in one descriptor set.
    #   xt[p, c*s + t] = x[n, t, c*128+p]   (fp8e3, 16 chunks of s)
    #   wv[p, et*128+d] = Wvsum[d, et*128+p]
    #   wo[p, j*E + o] = Wo[o, j*128+p]
    #   blk[p, rt*E + o] = block[rt*128+p, o]
    xt = nc.dram_tensor("xt", [128, et_n * s], F8, kind="ExternalInput").ap()
    wv = nc.dram_tensor("wv", [D, E], F16, kind="ExternalInput").ap()
    wo = nc.dram_tensor("wo", [128, et_n * E], F16, kind="ExternalInput").ap()
    blk = nc.dram_tensor("blk", [128, rt_n * E], F16, kind="ExternalOutput").ap()

    xp_n = et_n // 2         # x ships as 8 chunk-pairs of 1MB
    wc_n = et_n // 4         # wo ships as 4 quad-chunks of 2MB

    with tile.TileContext(nc) as tc:
        with (
            tc.tile_pool(name="wvp", bufs=1) as wvp,
            tc.tile_pool(name="xtp", bufs=8) as xtp,
            tc.tile_pool(name="wop", bufs=wc_n) as wop,
            tc.tile_pool(name="vsp", bufs=1) as vsp,
            tc.tile_pool(name="outp", bufs=2) as outp,
            tc.tile_pool(name="psp", bufs=4, space="PSUM") as psp,
        ):
            # junk tile for HAM warm-up matmuls (never DMA'd; memset once)
            junk = wvp.tile([128, 640], F16, name="junk")
            nc.gpsimd.memset(junk[:], 0.0)

            wv_sb = wvp.tile([128, E], F16, name="wv_sb")
            nc.sync.dma_start(wv_sb[:, 0:128], wv[:, 0:128])
            scr = wvp.tile([1, 2], F16, name="scr")
            nc.gpsimd.memset(scr[:], 0.0)
            nc.scalar.copy(scr[0:1, 1:2], scr[0:1, 0:1])  # prime ACT table early
            wv_t = [wv_sb[:, et * 128:(et + 1) * 128] for et in range(et_n)]

            vs = vsp.tile([128, s], F16)  # vsumT, [d, s]

            # stage-1 PSUM: 4 tiles x 2 banks (sc pairs); stage-2 reuses them.
            ps1 = [psp.tile([128, 1024], F32, tag="ps", name=f"ps1_{i}")
                   for i in range(4)]

            # HAM warm-up: PE busywork on junk during the initial DMA wait so
            # the clock un-throttles (~3.4us of contiguous activity) before
            # real matmuls.  N=256 keeps the granularity fine so the first
            # real matmul starts promptly once its DMA lands.
            for w in range(15):
                nc.tensor.matmul(ps1[0][:, 0:256], junk[:, 0:128],
                                 junk[:, 128:384], start=True, stop=True)

            # stage 1: vsumT accumulated over the 16 e-tiles (8 pairs).
            # x moves as fp8e3 against fp16 weights.
            xtiles = []
            for xp in range(xp_n):
                xtile = xtp.tile([128, 2 * s], F8, tag="xt", name=f"xt_{xp}")
                xtiles.append(xtile)
                base = xp * 2 * s
                if xp == 0:
                    # fine-grained pieces so the cold-start matmuls never
                    # out-run the DMA completions
                    nc.sync.dma_start(xtile[:, 0:512], xt[:, 0:512])
                    nc.sync.dma_start(xtile[:, 512:1536], xt[:, 512:1536])
                    nc.sync.dma_start(xtile[:, 1536:2560], xt[:, 1536:2560])
                    nc.sync.dma_start(xtile[:, 2560:s], xt[:, 2560:s])
                    nc.sync.dma_start(wv_sb[:, 128:E], wv[:, 128:E])
                    nc.sync.dma_start(xtile[:, s:2 * s], xt[:, s:2 * s])
                elif xp == 1:
                    nc.sync.dma_start(xtile[:, 0:s], xt[:, base:base + s])
                    nc.sync.dma_start(xtile[:, s:2 * s],
                                      xt[:, base + s:base + 2 * s])
                else:
                    nc.sync.dma_start(xtile[:], xt[:, base:base + 2 * s])
                for half_et in range(2):
                    et = 2 * xp + half_et
                    xv = xtile[:, half_et * s:(half_et + 1) * s]
                    for sc in range(sc_n):
                        nc.tensor.matmul(
                            ps1[sc // 2][:, (sc % 2) * 512:(sc % 2 + 1) * 512],
                            wv_t[et],
                            xv[:, sc * 512:(sc + 1) * 512],
                            start=(et == 0),
                            stop=(et == et_n - 1),
                        )
                    if et == 0:
                        # bridge the DMA-completion latency of the second
                        # e-tile's x so the HAM activity window stays busy.
                        # junk is all-zero, so accumulating into the open
                        # ps1[0] group adds exact zeros (start=False).
                        for w in range(4):
                            nc.tensor.matmul(ps1[0][:, 0:256], junk[:, 0:128],
                                             junk[:, 128:384],
                                             start=False, stop=False,
                                             skip_group_check=True)
            # wo DMAs: emitted now so the Sync engine issues them right after
            # the last x chunk; all of wo lands in SBUF well before stage 2
            # consumes it (no mid-stage-2 DMA dependency).
            wots = []
            for wc in range(wc_n):
                wot = wop.tile([128, 4 * E], F16, tag="wo", name=f"wo_{wc}")
                wots.append(wot)
                nc.sync.dma_start(wot[:], wo[:, wc * 4 * E:(wc + 1) * 4 * E])

            for sc in range(sc_n):
                dst = vs[:, sc * 512:(sc + 1) * 512]
                srcp = ps1[sc // 2][:, (sc % 2) * 512:(sc % 2 + 1) * 512]
                if sc % 2 == 0:
                    nc.vector.tensor_copy(dst, srcp)
                else:
                    nc.scalar.copy(dst, srcp)

            # boundary filler: keep PE's activity window busy while the vs
            # evacuation completes (prevents a HAM re-throttle at the stage
            # boundary).  ps1[0] has been read by its evac copies by then.
            for w in range(4):
                nc.tensor.matmul(ps1[0][:, 0:512], junk[:, 0:128],
                                 junk[:, 128:640], start=True, stop=True)

            # stage 2: block[r,o] = u @ Wo.T; lhsT j-tiles are stride-16
            # views of vsumT (u.T[t*128+d, r] = vsumT[d, r*16+t]).
            # PSUM: 4 tiles x 2 banks, [rt][oc-half]; reuse ps1's banks.
            ps2 = [psp.tile([128, 1024], F32, tag="ps", name=f"ps2_{i}")
                   for i in range(4)]
            ot = [outp.tile([128, E], F16, tag="out", name=f"out_{rt}")
                  for rt in range(rt_n)]
            for j in range(et_n):
                wot = wots[j // 4][:, (j % 4) * E:(j % 4 + 1) * E]
                last = (j == et_n - 1)
                for rt in range(rt_n):
                    base = rt * 2048 + j
                    lhsT = vs[:, base:base + 16 * 127 + 1:16]  # [128(d), 128(r)]
                    for oc in range(oc_n):
                        cs = slice(oc * 512, (oc + 1) * 512)
                        nc.tensor.matmul(
                            ps2[rt * 2 + oc // 2][:, (oc % 2) * 512:(oc % 2 + 1) * 512],
                            lhsT, wot[:, cs],
                            start=(j == 0), stop=last,
                        )
                        if last and oc % 2 == 1:
                            # evacuate each finished 2-bank tile while the
                            # remaining MMs still stream, then ship it
                            h = oc // 2
                            src = ps2[rt * 2 + h]
                            dc = slice(h * 1024, h * 1024 + 512)
                            dc2 = slice(h * 1024 + 512, (h + 1) * 1024)
                            nc.vector.tensor_copy(ot[rt][:, dc], src[:, 0:512])
                            nc.scalar.copy(ot[rt][:, dc2], src[:, 512:1024])
                            ds = slice(rt * E + h * 1024, rt * E + (h + 1) * 1024)
                            nc.sync.dma_start(blk[:, ds],
                                              ot[rt][:, h * 1024:(h + 1) * 1024])

    if not nc.is_finalized():
        nc.finalize()
    return nc


def kernel(x, Wq, Wk, Wv, Wo, bo):
    global last_results
    x = np.asarray(x, dtype=np.float32)
    Wv = np.asarray(Wv, dtype=np.float32)
    Wo = np.asarray(Wo, dtype=np.float32)
    bo = np.asarray(bo, dtype=np.float32)

    import ml_dtypes
    f8 = ml_dtypes.float8_e3m4
    bf = np.float16
    wvsum = Wv.reshape(H, D, E).sum(axis=0)            # [128, 2048] (d, e)
    wv16 = np.ascontiguousarray(                       # packed [128, 2048]
        wvsum.T.reshape(H, D, D).transpose(1, 0, 2).reshape(D, E)).astype(bf)
    # wo packed partition-major: wo16[p, j*E+o] = Wo[o, j*128+p]
    wo16 = np.ascontiguousarray(
        Wo.T.reshape(H, D, E).transpose(1, 0, 2).reshape(D, H * E)).astype(bf)
    # x packed partition-major: xt8[n][p, c*S+t] = x[n, t, c*128+p]
    xt8 = np.ascontiguousarray(
        np.clip(x, -15.0, 15.0).transpose(0, 2, 1)     # [8, E, S]
        .reshape(N, H, D, S).transpose(0, 2, 1, 3)     # [8, 128, 16, S]
        .reshape(N, D, H * S)).astype(f8)

    if S not in _prog_cache:
        _prog_cache[S] = _build_program(S)
    nc = _prog_cache[S]

    in_maps = [{"xt": xt8[n], "wv": wv16, "wo": wo16} for n in range(N_CORES)]
    _ensure_ntff_hook()
    try:
        last_results = run_bass_kernel_spmd(nc, in_maps, list(range(N_CORES)))
    except Exception:
        # If an externally-requested trace path (BASS_TRACE) fails in this
        # environment, fall back to a plain untraced run.
        if os.environ.get("BASS_TRACE") and not os.environ.get("BASS_NEVER_TRACE"):
            os.environ["BASS_NEVER_TRACE"] = "1"
            try:
                last_results = run_bass_kernel_spmd(nc, in_maps,
                                                    list(range(N_CORES)))
            finally:
                os.environ.pop("BASS_NEVER_TRACE", None)
        else:
            raise
    # blk dram layout [128, rt*E+o] -> block[rt*128+p, o]
    blocks = np.stack([
        last_results.results[n]["blk"].astype(np.float32)
        .reshape(D, R // D, E).transpose(1, 0, 2).reshape(R, E)
        for n in range(N_CORES)])

    out_block = blocks + bo[None, None, :]              # [8, 256, 2048]
    return np.tile(out_block, (1, H, 1)).astype(np.float32)


# revision 17
# speedup vs baseline: 1.0115x; 1.0115x over previous
"""Trainium2 Bass kernel for nn_MultiHeadAttention_32220844654809.

Mathematical simplification of the reference (faithful to its buggy einsum):

  q/k/v are projected then reshaped to [N,H,S,D].  The einsum
  'nqhd,nkhd->nqhk' contracts only d, so energy[n,hq,s,hk] is a 16x16
  head-head Gram matrix per (n,s); softmax is over hk.  The output einsum
  'nqhk,nvhd->nqhd' has BOTH k and v appearing in only one operand each,
  so it factorizes into (sum_k attention) * (sum_v v) = 1 * vsum, because
  softmax rows sum to one.  Q, K, Wq, Wk drop out entirely.

  vsum[n,s,d] = sum_h v[n,h,s,d] = x[n,s,:] @ Wvsum.T  with
  Wvsum[d,e] = sum_h Wv[h*128+d, e]  (head-summed V projection, [128,2048]).

  out.reshape(N,S,E) is a RAW reshape of the [N,H,S,D] tensor, and since the
  tensor is independent of the leading H axis, the [N,4096,2048] output is 16
  identical copies (along S) of block[n] = vsum[n].reshape(256,2048) @ Wo.T + bo.

Device work per core (data-parallel over batch N=8):
  stage 1: vsumT[d,s] = Wvsum @ x[n].T        (K=E=2048, M=128, N=4096)
  stage 2: block[r,o] = u @ Wo.T              (u = vsum.reshape(256,2048))
where u.T j-tiles are stride-16 views of vsumT in SBUF (no data movement).

Precision: x is shipped as float8_e3m4 (4 mantissa bits) and fed directly to
the PE as the moving operand against an fp16 stationary Wvsum (mixed-dtype
matmul upcasts both to ~fp22 internally).  This halves the dominant DMA
stream (x: 8.4MB instead of 16.8MB/core) so the kernel is PE-bound, while
keeping max|err|/absmax(ref) ~1.3e-2 < 2e-2 (weights/vsum/output stay fp16).
"""

import os
import sys
import types

import numpy as np

import concourse.mybir as mybir
import concourse.tile as tile
from concourse import bacc
from concourse.bass_utils import run_bass_kernel_spmd


def _ensure_ntff_hook():
    """If the image's antenv lacks axon_hooks, synthesize it so
    run_bass_kernel_spmd(trace=True) (e.g. via BASS_TRACE) degrades
    gracefully instead of raising ModuleNotFoundError."""
    try:
        import antenv.axon_hooks  # noqa: F401
        return
    except ImportError:
        pass
    try:
        import antenv
    except ImportError:
        return
    mod = types.ModuleType("antenv.axon_hooks")
    mod._hook = None
    mod.set_axon_ntff_profile_hook = lambda h: setattr(mod, "_hook", h)
    mod.get_axon_ntff_profile_hook = lambda: mod._hook
    sys.modules["antenv.axon_hooks"] = mod
    antenv.axon_hooks = mod
    try:
        from trn_agent_boot.trn_boot import _ntff_profile_via_ctypes
        if os.path.exists("/opt/axon/libaxon_pjrt.so"):
            mod._hook = _ntff_profile_via_ctypes("/opt/axon/libaxon_pjrt.so")
    except Exception:
        mod._hook = None

N_CORES = 8
N, S, E = 8, 4096, 2048
H, D = 16, 128
R = S // H  # 256 distinct output rows per batch; output = this block tiled 16x

F8 = mybir.dt.float8e3
F16 = mybir.dt.float16
F32 = mybir.dt.float32

_prog_cache = {}
last_results = None  # BassKernelResults of the most recent run (for test.py)


def _build_program(s=S):
    """One NeuronCore's program; run SPMD on 8 cores (core n <- batch n)."""
    et_n = E // 128          # 16 contraction tiles (e)
    sc_n = s // 512          # s-chunks of 512
    r = s // H               # block rows
    rt_n = r // 128          # output row tiles (2 at full size)
    oc_n = E // 512          # output col chunks (4)

    nc = bacc.Bacc("TRN2", target_bir_lowering=False, debug=False,
                   num_devices=N_CORES)
    # All DRAM tensors are packed partition-major on host (dim0 = SBUF
    # partition), flat in dim1, so every DMA moves multi-KB contiguous runs
    # per partition and pairs of e-tiles ship in one descriptor set.
    #   xt[p, c*s + t] = x[n, t, c*128+p]   (fp8e3, 16 chunks of s)
    #   wv[p, et*128+d] = Wvsum[d, et*128+p]
    #   wo[p, j*E + o] = Wo[o, j*128+p]
    #   blk[p, rt*E + o] = block[rt*128+p, o]
    xt = nc.dram_tensor("xt", [128, et_n * s], F8, kind="ExternalInput").ap()
    wv = nc.dram_tensor("wv", [D, E], F16, kind="ExternalInput").ap()
    wo = nc.dram_tensor("wo", [128, et_n * E], F16, kind="ExternalInput").ap()
    blk = nc.dram_tensor("blk", [128, rt_n * E], F16, kind="ExternalOutput").ap()

    xp_n = et_n // 2         # x ships as 8 chunk-pairs of 1MB
    wc_n = et_n // 4         # wo ships as 4 quad-chunks of 2MB

    with tile.TileContext(nc) as tc:
        with (
            tc.tile_pool(name="wvp", bufs=1) as wvp,
            tc.tile_pool(name="xtp", bufs=8) as xtp,
            tc.tile_pool(name="wop", bufs=wc_n) as wop,
            tc.tile_pool(name="vsp", bufs=1) as vsp,
            tc.tile_pool(name="outp", bufs=2) as outp,
            tc.tile_pool(name="psp", bufs=4, space="PSUM") as psp,
        ):
            # junk tile for HAM warm-up matmuls (never DMA'd; memset once)
            junk = wvp.tile([128, 640], F16, name="junk")
            nc.gpsimd.memset(junk[:], 0.0)

            wv_sb = wvp.tile([128, E], F16, name="wv_sb")
            nc.sync.dma_start(wv_sb[:, 0:128], wv[:, 0:128])
            scr = wvp.tile([1, 2], F16, name="scr")
            nc.gpsimd.memset(scr[:], 0.0)
            nc.scalar.copy(scr[0:1, 1:2], scr[0:1, 0:1])  # prime ACT table early
            wv_t = [wv_sb[:, et * 128:(et + 1) * 128] for et in range(et_n)]

            vs = vsp.tile([128, s], F16)  # vsumT, [d, s]

            # stage-1 PSUM: 4 tiles x 2 banks (sc pairs); stage-2 reuses them.
            ps1 = [psp.tile([128, 1024], F32, tag="ps", name=f"ps1_{i}")
                   for i in range(4)]

            # HAM warm-up: PE busywork on junk while the first x pieces and
            # weights stream in (~5us).  Sized so the clock un-throttles
            # (~3.4us of contiguous activity) AND the whole first x pair has
            # landed before the real matmuls begin -- the real stream then
            # runs warm and gap-free, immune to DMA completion jitter.
            for w in range(40):
                nc.tensor.matmul(ps1[0][:, 0:256], junk[:, 0:128],
                                 junk[:, 128:384], start=True, stop=True)

            # stage 1: vsumT accumulated over the 16 e-tiles (8 pairs).
            # x moves as fp8e3 against fp16 weights.
            xtiles = []
            for xp in range(xp_n):
                xtile = xtp.tile([128, 2 * s], F8, tag="xt", name=f"xt_{xp}")
                xtiles.append(xtile)
                base = xp * 2 * s
                if xp == 0:
                    # fine-grained pieces; all land during the warm-up block
                    nc.sync.dma_start(xtile[:, 0:512], xt[:, 0:512])
                    nc.sync.dma_start(wv_sb[:, 128:E], wv[:, 128:E])
                    nc.sync.dma_start(xtile[:, 512:1536], xt[:, 512:1536])
                    nc.sync.dma_start(xtile[:, 1536:2560], xt[:, 1536:2560])
                    nc.sync.dma_start(xtile[:, 2560:s], xt[:, 2560:s])
                    nc.sync.dma_start(xtile[:, s:2 * s], xt[:, s:2 * s])
                elif xp == 1:
                    nc.sync.dma_start(xtile[:, 0:s], xt[:, base:base + s])
                    nc.sync.dma_start(xtile[:, s:2 * s],
                                      xt[:, base + s:base + 2 * s])
                else:
                    nc.sync.dma_start(xtile[:], xt[:, base:base + 2 * s])
                for half_et in range(2):
                    et = 2 * xp + half_et
                    xv = xtile[:, half_et * s:(half_et + 1) * s]
                    for sc in range(sc_n):
                        nc.tensor.matmul(
                            ps1[sc // 2][:, (sc % 2) * 512:(sc % 2 + 1) * 512],
                            wv_t[et],
                            xv[:, sc * 512:(sc + 1) * 512],
                            start=(et == 0),
                            stop=(et == et_n - 1),
                        )

            # wo DMAs: emitted now so the Sync engine issues them right after
            # the last x chunk; all of wo lands in SBUF well before stage 2
            # consumes it (no mid-stage-2 DMA dependency).
            wots = []
            for wc in range(wc_n):
                wot = wop.tile([128, 4 * E], F16, tag="wo", name=f"wo_{wc}")
                wots.append(wot)
                nc.sync.dma_start(wot[:], wo[:, wc * 4 * E:(wc + 1) * 4 * E])

            for sc in range(sc_n):
                dst = vs[:, sc * 512:(sc + 1) * 512]
                srcp = ps1[sc // 2][:, (sc % 2) * 512:(sc % 2 + 1) * 512]
                if sc % 2 == 0:
                    nc.vector.tensor_copy(dst, srcp)
                else:
                    nc.scalar.copy(dst, srcp)

            # boundary filler: keep PE's activity window busy while the vs
            # evacuation completes (prevents a HAM re-throttle at the stage
            # boundary).  ps1[0] has been read by its evac copies by then.
            for w in range(4):
                nc.tensor.matmul(ps1[0][:, 0:512], junk[:, 0:128],
                                 junk[:, 128:640], start=True, stop=True)

            # stage 2: block[r,o] = u @ Wo.T; lhsT j-tiles are stride-16
            # views of vsumT (u.T[t*128+d, r] = vsumT[d, r*16+t]).
            # PSUM: 4 tiles x 2 banks, [rt][oc-half]; reuse ps1's banks.
            ps2 = [psp.tile([128, 1024], F32, tag="ps", name=f"ps2_{i}")
                   for i in range(4)]
            ot = [outp.tile([128, E], F16, tag="out", name=f"out_{rt}")
                  for rt in range(rt_n)]
            for j in range(et_n):
                wot = wots[j // 4][:, (j % 4) * E:(j % 4 + 1) * E]
                last = (j == et_n - 1)
                for rt in range(rt_n):
                    base = rt * 2048 + j
                    lhsT = vs[:, base:base + 16 * 127 + 1:16]  # [128(d), 128(r)]
                    for oc in range(oc_n):
                        cs = slice(oc * 512, (oc + 1) * 512)
                        nc.tensor.matmul(
                            ps2[rt * 2 + oc // 2][:, (oc % 2) * 512:(oc % 2 + 1) * 512],
                            lhsT, wot[:, cs],
                            start=(j == 0), stop=last,
                        )
                        if last and oc % 2 == 1:
                            # evacuate each finished 2-bank tile while the
                            # remaining MMs still stream, then ship it
                            h = oc // 2
                            src = ps2[rt * 2 + h]
                            dc = slice(h * 1024, h * 1024 + 512)
                            dc2 = slice(h * 1024 + 512, (h + 1) * 1024)
                            nc.vector.tensor_copy(ot[rt][:, dc], src[:, 0:512])
                            nc.scalar.copy(ot[rt][:, dc2], src[:, 512:1024])
                            ds = slice(rt * E + h * 1024, rt * E + (h + 1) * 1024)
                            nc.sync.dma_start(blk[:, ds],
                                              ot[rt][:, h * 1024:(h + 1) * 1024])

    if not nc.is_finalized():
        nc.finalize()
    return nc


def kernel(x, Wq, Wk, Wv, Wo, bo):
    global last_results
    x = np.asarray(x, dtype=np.float32)
    Wv = np.asarray(Wv, dtype=np.float32)
    Wo = np.asarray(Wo, dtype=np.float32)
    bo = np.asarray(bo, dtype=np.float32)

    import ml_dtypes
    f8 = ml_dtypes.float8_e3m4
    bf = np.float16
    wvsum = Wv.reshape(H, D, E).sum(axis=0)            # [128, 2048] (d, e)
    wv16 = np.ascontiguousarray(                       # packed [128, 2048]
        wvsum.T.reshape(H, D, D).transpose(1, 0, 2).reshape(D, E)).astype(bf)
    # wo packed partition-major: wo16[p, j*E+o] = Wo[o, j*128+p]
    wo16 = np.ascontiguousarray(
        Wo.T.reshape(H, D, E).transpose(1, 0, 2).reshape(D, H * E)).astype(bf)
    # x packed partition-major: xt8[n][p, c*S+t] = x[n, t, c*128+p]
    xt8 = np.ascontiguousarray(
        np.clip(x, -15.0, 15.0).transpose(0, 2, 1)     # [8, E, S]
        .reshape(N, H, D, S).transpose(0, 2, 1, 3)     # [8, 128, 16, S]
        .reshape(N, D, H * S)).astype(f8)

    if S not in _prog_cache:
        _prog_cache[S] = _build_program(S)
    nc = _prog_cache[S]

    in_maps = [{"xt": xt8[n], "wv": wv16, "wo": wo16} for n in range(N_CORES)]
    _ensure_ntff_hook()
    try:
        last_results = run_bass_kernel_spmd(nc, in_maps, list(range(N_CORES)))
    except Exception:
        # If an externally-requested trace path (BASS_TRACE) fails in this
        # environment, fall back to a plain untraced run.
        if os.environ.get("BASS_TRACE") and not os.environ.get("BASS_NEVER_TRACE"):
            os.environ["BASS_NEVER_TRACE"] = "1"
            try:
                last_results = run_bass_kernel_spmd(nc, in_maps,
                                                    list(range(N_CORES)))
            finally:
                os.environ.pop("BASS_NEVER_TRACE", None)
        else:
            raise
    # blk dram layout [128, rt*E+o] -> block[rt*128+p, o]
    blocks = np.stack([
        last_results.results[n]["blk"].astype(np.float32)
        .reshape(D, R // D, E).transpose(1, 0, 2).reshape(R, E)
        for n in range(N_CORES)])

    out_block = blocks + bo[None, None, :]              # [8, 256, 2048]
    return np.tile(out_block, (1, H, 1)).astype(np.float32)
